# revision 55
# baseline (speedup 1.0000x reference)
"""Multi-head causal attention (B=1, S=2048, E=2048, H=16, DH=128) on 8 TRN2
NeuronCores.

Sharding: tensor-parallel over heads; core c owns heads 2c and 2c+1; output
projection column-sharded (core c computes y[:, 256c:256(c+1)]) after an
AllGather of the per-group attention outputs.

v3 schedule: projections and attention are interleaved per q-group so the
AllGathers start ~90us earlier and finish well before the PE runs dry:

  pass0 (stream x^T): Q(g0) K(g0) both heads + V blocks 0-3
  attn(0) -> AG0
  pass1: Q(g1) K(g1) + V 4-7;   attn(1) -> AG1
  pass2: Q(g2) K(g2) + V 8-11;  attn(2) -> AG2
  pass3: Q(g3) K(g3) + V 12-15; og0 prefetch; attn(3) -> AG3
  og1-3 prefetch; output-projection tails fill the PE during the gathers.

Each pass's QK chains and V chains read the same x^T column range (group g),
so pass0 chases the x^T stream tile by tile (kt=0 split in 4 chunks so the
first matmul starts right after ~200KB of DMA).

All operands bf16 (fp32 PSUM accumulation), og tiles load with one batched
DMA per group issued from the Scalar queue, denominators use
reciprocal_approx_fast, y stores are one batched DMA per group.
PSUM budget: proj passes 4 QK + 4 V accs = 8 banks; attention
psS 3 + psO 3 + psN 2 = 8 banks; tails rotate 4 banks.
"""
import os
import sys
from contextlib import ExitStack

if "/opt/trn_rl_repo" not in sys.path:
    sys.path.insert(0, "/opt/trn_rl_repo")

import numpy as np

B, S, E, H = 1, 2048, 2048, 16
DH = E // H          # 128
N_CORES = 8
HPC = H // N_CORES   # heads per core = 2
KT = E // 128        # 16 contraction tiles
QG = 512             # q-group width
SBK = S // 128       # 16 s/sk blocks
CSL = E // N_CORES   # 256 output columns per core

# (q0, qw, nj): query start, width, number of 128-key blocks attended
GROUPS = [(0, 512, 4), (512, 512, 8), (1024, 512, 12), (1536, 512, 16)]
NG = len(GROUPS)

_CACHE = {}


def _build(fp_name: str):
    import concourse.bass as bass  # noqa: F401
    import concourse.bass_isa as bass_isa
    import concourse.mybir as mybir
    import concourse.tile as tile
    from concourse import bacc

    FP = getattr(mybir.dt, fp_name)
    F32 = mybir.dt.float32
    BF16 = mybir.dt.bfloat16
    AF = mybir.ActivationFunctionType

    nc = bacc.Bacc("TRN2", target_bir_lowering=False, debug=False,
                   num_devices=N_CORES)

    xT_t = nc.dram_tensor("xT", [E, S], BF16, kind="ExternalInput")
    wq_t = nc.dram_tensor("wq", [128, KT * HPC * DH], BF16, kind="ExternalInput")
    wk_t = nc.dram_tensor("wk", [128, KT * HPC * DH], BF16, kind="ExternalInput")
    wv_t = nc.dram_tensor("wv", [128, KT * HPC * DH], BF16, kind="ExternalInput")
    bq_t = nc.dram_tensor("bq", [DH, HPC], F32, kind="ExternalInput")
    bk_t = nc.dram_tensor("bk", [DH, HPC], F32, kind="ExternalInput")
    bv_t = nc.dram_tensor("bv", [1, HPC * DH], F32, kind="ExternalInput")
    wo_t = nc.dram_tensor("wo", [128, KT * CSL], BF16, kind="ExternalInput")
    bo_t = nc.dram_tensor("bo", [1, CSL], F32, kind="ExternalInput")
    mask_t = nc.dram_tensor("mask", [4 * 128, QG], BF16, kind="ExternalInput")
    y_t = nc.dram_tensor("y", [S, CSL], F32, kind="ExternalOutput")

    xT_r = xT_t.ap().rearrange("(kt p) s -> kt p s", p=128)
    mask_r = mask_t.ap().rearrange("(jm p) q -> jm p q", p=128)

    scale = 1.0 / float(np.sqrt(DH))

    with tile.TileContext(nc) as tc:
        with tc.tile_pool(name="const", bufs=1) as constp, \
             tc.tile_pool(name="prod", bufs=1) as prodp, \
             tc.tile_pool(name="dram", bufs=1, space="DRAM") as dramp:
            # Weight staging. Full-tensor DMAs only: column-sliced weight
            # loads fragment into 128x512B descriptors and crawl on a cold
            # queue. Sync carries wq(h0) then the x^T stream; everything else
            # rides the Scalar HWDGE queue in first-use order, with the
            # fragmented small transfers (biases, masks) last.
            wqk_sb = {}
            for nm_ in ("wq", "wk"):
                for hh in range(HPC):
                    wqk_sb[(nm_, hh)] = constp.tile(
                        [128, KT * DH], BF16,
                        tag=f"w_{nm_}{hh}", name=f"w_{nm_}{hh}")
            HKD = KT * DH // 2
            nc.sync.dma_start(wqk_sb[("wq", 0)][:, 0:HKD],
                              wq_t.ap()[:, 0:HKD])
            nc.sync.dma_start(wqk_sb[("wq", 1)][:, 0:HKD],
                              wq_t.ap()[:, KT * DH:KT * DH + HKD])
            nc.scalar.dma_start(wqk_sb[("wk", 0)][:, 0:HKD],
                                wk_t.ap()[:, 0:HKD])
            nc.scalar.dma_start(wqk_sb[("wk", 1)][:, 0:HKD],
                                wk_t.ap()[:, KT * DH:KT * DH + HKD])
            nc.scalar.dma_start(wqk_sb[("wk", 0)][:, HKD:KT * DH],
                                wk_t.ap()[:, HKD:KT * DH])
            nc.scalar.dma_start(wqk_sb[("wk", 1)][:, HKD:KT * DH],
                                wk_t.ap()[:, KT * DH + HKD:2 * KT * DH])
            wv_sb = constp.tile([128, KT * HPC * DH], BF16, tag="wv_sb",
                                name="wv_sb")
            wos = constp.tile([128, KT * CSL], BF16, tag="wos", name="wos")
            nc.scalar.dma_start(wv_sb[:], wv_t.ap()[:])
            nc.scalar.dma_start(wos[:], wo_t.ap()[:])
            bqs = constp.tile([DH, HPC], F32)
            nc.scalar.dma_start(bqs[:], bq_t.ap()[:])
            bks = constp.tile([DH, HPC], F32)
            nc.scalar.dma_start(bks[:], bk_t.ap()[:])
            masks = constp.tile([128, 4 * QG], BF16)
            for jm in range(4):
                nc.scalar.dma_start(masks[:, jm * QG:(jm + 1) * QG],
                                    mask_r[jm])
            bvs = constp.tile([128, HPC * DH], F32)
            bos = constp.tile([128, CSL], F32)
            nc.scalar.dma_start(
                bvs[:], bv_t.ap().to_broadcast((128, HPC * DH)))
            nc.scalar.dma_start(bos[:], bo_t.ap().to_broadcast((128, CSL)))
            ones_f32 = constp.tile([128, 128], F32)
            nc.vector.memset(ones_f32[:], 1.0)
            ones_col = constp.tile([128, 1], FP)
            nc.vector.tensor_copy(ones_col[:], ones_f32[:, 0:1])


            # --- products ---
            qkt = prodp.tile([128, HPC * S], FP)   # Q^T, head hh at cols hh*S
            kkt = prodp.tile([128, HPC * S], FP)   # K^T
            vt = prodp.tile([128, SBK * HPC * DH], FP)  # V, s-block sb at sb*256

            cin = [dramp.tile([HPC * DH, qw], BF16, tag=f"cin{g}",
                              name=f"cin{g}")
                   for g, (q0, qw, nj) in enumerate(GROUPS)]
            cout = [dramp.tile([N_CORES, HPC * DH, qw], BF16,
                               tag=f"cout{g}", name=f"cout{g}",
                               addr_space="Shared")
                    for g, (q0, qw, nj) in enumerate(GROUPS)]

            psum_stack = ExitStack()
            with tc.tile_pool(name="osb", bufs=1) as osbp, \
                 tc.tile_pool(name="pt", bufs=8) as ptp, \
                 tc.tile_pool(name="rec", bufs=2) as recp, \
                 tc.tile_pool(name="bcs", bufs=2) as bcsp:
                o_sbuf = osbp.tile([128, HPC * S], BF16)

                pp = {}   # persistent PSUM pools, filled after the stream

                def attn(gi, split_at=None, between=None):
                    q0, qw, nj = GROUPS[gi]
                    npairs = nj // 2
                    jmax = nj - 1
                    mask_start = q0 // 128     # first masked j-block
                    jm0 = (512 * (q0 // 512)) // 128
                    col_off = q0 - 512 * (q0 // 512)
                    psS, psO, psN = pp["psS"], pp["psO"], pp["psN"]
                    o_acc = [psO.tile([128, QG], F32, tag="o",
                                      name=f"o{hh}") for hh in range(HPC)]
                    s_acc = [psN.tile([1, QG], F32, tag="n",
                                      name=f"n{hh}") for hh in range(HPC)]

                    def emit_pv(hh, jp, pt):
                        for dj in range(2):
                            j = 2 * jp + dj
                            nc.tensor.matmul(
                                o_acc[hh][:, 0:qw],
                                vt[:, j * HPC * DH + hh * DH:
                                   j * HPC * DH + (hh + 1) * DH],
                                pt[:, dj * qw:(dj + 1) * qw],
                                start=(j == 0), stop=(j == jmax))
                            nc.tensor.matmul(
                                s_acc[hh][:, 0:qw],
                                ones_col[:],
                                pt[:, dj * qw:(dj + 1) * qw],
                                start=(j == 0), stop=(j == jmax))

                    recs = {}

                    pend = []

                    def pop_pend():
                        hh, jp, pt = pend.pop(0)
                        emit_pv(hh, jp, pt)
                        if jp == npairs - 1:
                            # reciprocal on DVE as soon as this head's
                            # denominator chain stops
                            rec = recp.tile([1, QG], F32, tag="r",
                                            name="rec")
                            nc.vector.reciprocal_approx_fast(
                                rec[:, 0:qw], s_acc[hh][:, 0:qw])
                            recs[hh] = rec

                    for jp in range(npairs):
                        if jp == split_at:
                            # flush, then emit interposed projection work;
                            # the PSUM accumulation chains stay open
                            while pend:
                                pop_pend()
                            between()
                        for hh in range(HPC):
                            pt = ptp.tile([128, 2 * QG], FP, tag="p",
                                          name="pt")
                            for dj in range(2):
                                j = 2 * jp + dj
                                ps = psS.tile([128, QG], F32, tag="s",
                                              name="ps")
                                nc.tensor.matmul(
                                    ps[:, 0:qw],
                                    kkt[:, hh * S + j * 128:
                                        hh * S + (j + 1) * 128],
                                    qkt[:, hh * S + q0:
                                        hh * S + q0 + qw],
                                    start=True, stop=True)
                                nc.scalar.activation(
                                    pt[:, dj * qw:(dj + 1) * qw],
                                    ps[:, 0:qw], AF.Exp, scale=scale)
                                if j >= mask_start:
                                    jm = j - jm0
                                    nc.vector.tensor_mul(
                                        pt[:, dj * qw:(dj + 1) * qw],
                                        pt[:, dj * qw:(dj + 1) * qw],
                                        masks[:, jm * QG + col_off:
                                              jm * QG + col_off + qw])
                            pend.append((hh, jp, pt))
                            while len(pend) > 3:
                                pop_pend()
                    while pend:
                        pop_pend()

                    for hh in range(HPC):
                        bcs = bcsp.tile([128, QG], F32, tag="b",
                                        name="bcs")
                        nc.gpsimd.partition_broadcast(
                            bcs[:, 0:qw], recs[hh][:, 0:qw])
                        nc.vector.tensor_mul(
                            o_sbuf[:, hh * S + q0:hh * S + q0 + qw],
                            o_acc[hh][:, 0:qw], bcs[:, 0:qw])
                        nc.sync.dma_start(
                            cin[gi].rearrange("(hh p) q -> hh p q",
                                              p=128)[hh],
                            o_sbuf[:, hh * S + q0:hh * S + q0 + qw])
                    nc.gpsimd.collective_compute(
                        "AllGather",
                        mybir.AluOpType.bypass,
                        replica_groups=[list(range(N_CORES))],
                        ins=[cin[gi].opt()],
                        outs=[cout[gi].opt()],
                    )

                # ===== interleaved projections + attention =====
                with tc.tile_pool(name="xt", bufs=1) as xtp:
                    xt = xtp.tile([128, KT * S], BF16)

                    def qk_specs(groups):
                        # drain order: everything attn(groups[0]) needs first
                        return [(wn, prod, bias, hh, g)
                                for g in groups
                                for hh in range(HPC)
                                for (wn, prod, bias) in
                                (("wq", qkt, bqs), ("wk", kkt, bks))]

                    def proj_stream(groups, tag):
                        """Stream x^T; QK chains for `groups` chase it."""
                        specs = qk_specs(groups)
                        with tc.tile_pool(name=f"psP{tag}", bufs=1,
                                          space="PSUM") as psA:
                            accs = [psA.tile([128, QG], F32, tag="qk",
                                             name=f"qk{tag}_{i}",
                                             bufs=len(specs))
                                    for i in range(len(specs))]
                            HKD = KT * DH // 2
                            for kt in range(KT):
                                # alternate queues so the stream never
                                # starves the PE (gpsimd is idle here)
                                eng = nc.sync if kt % 2 == 0 else nc.gpsimd
                                eng.dma_start(
                                    xt[:, kt * S:(kt + 1) * S], xT_r[kt])
                                if kt == 0:
                                    # second wq halves behind the first tile
                                    nc.sync.dma_start(
                                        wqk_sb[("wq", 0)][:, HKD:KT * DH],
                                        wq_t.ap()[:, HKD:KT * DH])
                                    nc.sync.dma_start(
                                        wqk_sb[("wq", 1)][:, HKD:KT * DH],
                                        wq_t.ap()[:, KT * DH + HKD:
                                                  2 * KT * DH])
                                for i, (wn, prod, bias, hh, g) in \
                                        enumerate(specs):
                                    nc.tensor.matmul(
                                        accs[i][:],
                                        wqk_sb[(wn, hh)][:, kt * DH:
                                                         (kt + 1) * DH],
                                        xt[:, kt * S + g * QG:
                                           kt * S + (g + 1) * QG],
                                        start=(kt == 0), stop=(kt == KT - 1))
                            for i, (wn, prod, bias, hh, g) in \
                                    enumerate(specs):
                                # alternate drains between Scalar and DVE to
                                # halve the serial drain tail
                                if i % 2 == 0:
                                    nc.scalar.activation(
                                        prod[:, hh * S + g * QG:
                                             hh * S + (g + 1) * QG],
                                        accs[i][:], AF.Identity,
                                        bias=bias[:, hh:hh + 1])
                                else:
                                    nc.vector.tensor_scalar_add(
                                        prod[:, hh * S + g * QG:
                                             hh * S + (g + 1) * QG],
                                        accs[i][:], bias[:, hh:hh + 1])

                    def proj_rest(specs, vbs, tag):
                        """QK chains from `specs` + V chains for blocks vbs,
                        resident x^T, chain-major so drains overlap. Accs
                        rotate through the persistent 2-slot acc pool."""
                        psA = pp["psA"]
                        for i, (wn, prod, bias, hh, g) in enumerate(specs):
                            acc = psA.tile([128, QG], F32, tag="acc",
                                           name=f"qk{tag}_{i}")
                            for kt in range(KT):
                                nc.tensor.matmul(
                                    acc[:],
                                    wqk_sb[(wn, hh)][:, kt * DH:
                                                     (kt + 1) * DH],
                                    xt[:, kt * S + g * QG:
                                       kt * S + (g + 1) * QG],
                                    start=(kt == 0), stop=(kt == KT - 1))
                            if i % 2 == 0:
                                nc.scalar.activation(
                                    prod[:, hh * S + g * QG:
                                         hh * S + (g + 1) * QG],
                                    acc[:], AF.Identity,
                                    bias=bias[:, hh:hh + 1])
                            else:
                                nc.vector.tensor_scalar_add(
                                    prod[:, hh * S + g * QG:
                                         hh * S + (g + 1) * QG],
                                    acc[:], bias[:, hh:hh + 1])
                        for bi, b in enumerate(vbs):
                            acc = psA.tile([128, QG], F32, tag="acc",
                                           name=f"v{tag}_{b}")
                            for kt in range(KT):
                                nc.tensor.matmul(
                                    acc[:, 0:HPC * DH],
                                    xt[:, kt * S + b * 128:
                                       kt * S + (b + 1) * 128],
                                    wv_sb[:, kt * HPC * DH:
                                          (kt + 1) * HPC * DH],
                                    start=(kt == 0), stop=(kt == KT - 1))
                            nc.vector.tensor_add(
                                vt[:, b * HPC * DH:(b + 1) * HPC * DH],
                                acc[:, 0:HPC * DH], bvs[:])

                    proj_stream([0, 1], "s0")          # QK g0+g1, 8 chains
                    # persistent PSUM pools for the rest of the kernel: no
                    # pool-close barriers between phases (8 banks total)
                    pp["psS"] = psum_stack.enter_context(
                        tc.tile_pool(name="psS", bufs=2, space="PSUM"))
                    pp["psO"] = psum_stack.enter_context(
                        tc.tile_pool(name="psO", bufs=2, space="PSUM"))
                    pp["psN"] = psum_stack.enter_context(
                        tc.tile_pool(name="psN", bufs=2, space="PSUM"))
                    pp["psA"] = psum_stack.enter_context(
                        tc.tile_pool(name="psA", bufs=2, space="PSUM"))
                    proj_rest([], [0, 1, 2, 3, 4, 5, 6, 7], "s0b")
                    attn(0)
                    # pass1 also projects Q(g3) so most of attn(3) can run
                    # before the K(g3)/V(12-15) pass
                    proj_rest(qk_specs([2]) +
                              [("wq", qkt, bqs, hh, 3) for hh in range(HPC)],
                              [8, 9, 10, 11], "s1")
                    attn(1)
                    attn(2)
                    attn(3, split_at=6, between=lambda: proj_rest(
                        [("wk", kkt, bks, hh, 3) for hh in range(HPC)],
                        [12, 13, 14, 15], "s2"))
                # xt pool closed: 64KB/partition freed for og prefetch
                with tc.tile_pool(name="og", bufs=4) as ogp:
                    og = {}

                    def og_load(gi):
                        q0, qw, nj = GROUPS[gi]
                        t = ogp.tile([128, KT * QG], BF16, tag="og",
                                     name=f"og{gi}")
                        og[gi] = t
                        # kt-quarters so the tail chain starts on the first
                        # quarter while the rest transfers. Sync queue: its
                        # only later work is the y stores, which trail the
                        # og-gated tails anyway.
                        Q4 = KT * qw // 4
                        for ci in range(4):
                            lo, hi = ci * Q4, (ci + 1) * Q4
                            nc.sync.dma_start(
                                t[:, lo:hi].rearrange(
                                    "p (c h q) -> p c h q",
                                    c=N_CORES // 4, h=HPC),
                                cout[gi].rearrange(
                                    "c (h p) q -> p c h q",
                                    p=128)[:, lo // (HPC * qw):
                                           hi // (HPC * qw)])

                    for gi in range(NG):
                        og_load(gi)

                    # ===== tail: output projection, column-sharded =====
                    with tc.tile_pool(name="yst", bufs=2) as ystp:
                        for gi, (q0, qw, nj) in enumerate(GROUPS):
                            nsb = qw // 128
                            yst = ystp.tile([128, 4 * CSL], F32, tag="ys",
                                            name=f"yst{gi}")
                            for i in range(nsb):
                                acc = pp["psA"].tile([128, QG], F32,
                                                     tag="acc", name="yacc")
                                for kt in range(KT):
                                    nc.tensor.matmul(
                                        acc[:, 0:CSL],
                                        og[gi][:, kt * qw + i * 128:
                                               kt * qw + (i + 1) * 128],
                                        wos[:, kt * CSL:(kt + 1) * CSL],
                                        start=(kt == 0), stop=(kt == KT - 1))
                                nc.vector.tensor_add(
                                    yst[:, i * CSL:(i + 1) * CSL],
                                    acc[:, 0:CSL], bos[:])
                            nc.sync.dma_start(
                                y_t.ap()[q0:q0 + qw, :].rearrange(
                                    "(sb p) c -> p sb c", p=128),
                                yst[:, 0:nsb * CSL].rearrange(
                                    "p (sb c) -> p sb c", c=CSL))
                psum_stack.close()

    nc.compile()
    return nc


def _tilize(w):
    """[E, cols] -> [128, KT*cols]: k-tile kt at columns kt*cols."""
    cols = w.shape[1]
    return np.ascontiguousarray(
        w.reshape(KT, 128, cols).transpose(1, 0, 2).reshape(128, KT * cols))


def _tilize_hm(w):
    """[E, HPC*DH] -> [128, HPC*KT*DH], head-major then k-tile."""
    return np.ascontiguousarray(
        w.reshape(KT, 128, HPC, DH).transpose(1, 2, 0, 3)
        .reshape(128, HPC * KT * DH))


def _prep_inputs(x, Wq, bq, Wk, bk, Wv, bv, WO, bo):
    import ml_dtypes

    f32 = np.float32
    bf16 = ml_dtypes.bfloat16
    xT = np.ascontiguousarray(np.asarray(x, f32)[0].T).astype(bf16)
    Wq = np.asarray(Wq, f32); Wk = np.asarray(Wk, f32); Wv = np.asarray(Wv, f32)
    bq = np.asarray(bq, f32); bk = np.asarray(bk, f32); bv = np.asarray(bv, f32)
    WO = np.asarray(WO, f32); bo = np.asarray(bo, f32)

    jm = np.arange(4)[:, None, None]
    r = np.arange(128)[None, :, None]
    c = np.arange(QG)[None, None, :]
    mask = (128 * jm + r <= c).astype(bf16).reshape(4 * 128, QG)

    in_maps = []
    for cidx in range(N_CORES):
        h0, h1 = HPC * cidx, HPC * cidx + 1
        in_maps.append({
            "xT": xT,
            "wq": _tilize_hm(np.concatenate([Wq[h0], Wq[h1]], 1)).astype(bf16),
            "wk": _tilize_hm(np.concatenate([Wk[h0], Wk[h1]], 1)).astype(bf16),
            "wv": _tilize(np.concatenate([Wv[h0], Wv[h1]], 1)).astype(bf16),
            "bq": np.ascontiguousarray(np.stack([bq[h0], bq[h1]], 1)),
            "bk": np.ascontiguousarray(np.stack([bk[h0], bk[h1]], 1)),
            "bv": np.concatenate([bv[h0], bv[h1]])[None, :].copy(),
            "wo": _tilize(np.ascontiguousarray(
                WO[:, CSL * cidx:CSL * (cidx + 1)])).astype(bf16),
            "bo": bo[CSL * cidx:CSL * (cidx + 1)][None, :].copy(),
            "mask": mask,
        })
    return in_maps


def kernel(x, Wq, bq, Wk, bk, Wv, bv, WO, bo, trace=False,
           fp_name="bfloat16"):
    from concourse.bass_utils import run_bass_kernel_spmd

    key = fp_name
    if key not in _CACHE:
        _CACHE[key] = _build(fp_name)
    nc = _CACHE[key]

    in_maps = _prep_inputs(x, Wq, bq, Wk, bk, Wv, bv, WO, bo)
    kwargs = {}
    if trace:
        kwargs["trace"] = True
    res = run_bass_kernel_spmd(nc, in_maps, core_ids=list(range(N_CORES)),
                               **kwargs)
    kernel.last_results = res

    y = np.concatenate([res.results[c]["y"] for c in range(N_CORES)], axis=1)
    return y.reshape(B, S, E).astype(np.float32)


# revision 60
# speedup vs baseline: 1.0822x; 1.0822x over previous
"""Multi-head causal attention (B=1, S=2048, E=2048, H=16, DH=128) on 8 TRN2
NeuronCores.

Sharding: tensor-parallel over heads; core c owns heads 2c and 2c+1; output
projection column-sharded (core c computes y[:, 256c:256(c+1)]) after an
AllGather of the per-group attention outputs.

v3 schedule: projections and attention are interleaved per q-group so the
AllGathers start ~90us earlier and finish well before the PE runs dry:

  pass0 (stream x^T): Q(g0) K(g0) both heads + V blocks 0-3
  attn(0) -> AG0
  pass1: Q(g1) K(g1) + V 4-7;   attn(1) -> AG1
  pass2: Q(g2) K(g2) + V 8-11;  attn(2) -> AG2
  pass3: Q(g3) K(g3) + V 12-15; og0 prefetch; attn(3) -> AG3
  og1-3 prefetch; output-projection tails fill the PE during the gathers.

Each pass's QK chains and V chains read the same x^T column range (group g),
so pass0 chases the x^T stream tile by tile (kt=0 split in 4 chunks so the
first matmul starts right after ~200KB of DMA).

All operands bf16 (fp32 PSUM accumulation), og tiles load with one batched
DMA per group issued from the Scalar queue, denominators use
reciprocal_approx_fast, y stores are one batched DMA per group.
PSUM budget: proj passes 4 QK + 4 V accs = 8 banks; attention
psS 3 + psO 3 + psN 2 = 8 banks; tails rotate 4 banks.
"""
import os
import sys
from contextlib import ExitStack

if "/opt/trn_rl_repo" not in sys.path:
    sys.path.insert(0, "/opt/trn_rl_repo")

import numpy as np

B, S, E, H = 1, 2048, 2048, 16
DH = E // H          # 128
N_CORES = 8
HPC = H // N_CORES   # heads per core = 2
KT = E // 128        # 16 contraction tiles
QG = 512             # q-group width
SBK = S // 128       # 16 s/sk blocks
CSL = E // N_CORES   # 256 output columns per core

# (q0, qw, nj): query start, width, number of 128-key blocks attended
GROUPS = [(0, 512, 4), (512, 512, 8), (1024, 512, 12), (1536, 512, 16)]
NG = len(GROUPS)

_CACHE = {}


def _build(fp_name: str):
    import concourse.bass as bass  # noqa: F401
    import concourse.bass_isa as bass_isa
    import concourse.mybir as mybir
    import concourse.tile as tile
    from concourse import bacc

    FP = getattr(mybir.dt, fp_name)
    F32 = mybir.dt.float32
    BF16 = mybir.dt.bfloat16
    AF = mybir.ActivationFunctionType

    nc = bacc.Bacc("TRN2", target_bir_lowering=False, debug=False,
                   num_devices=N_CORES)

    xT_t = nc.dram_tensor("xT", [E, S], BF16, kind="ExternalInput")
    wq_t = nc.dram_tensor("wq", [128, KT * HPC * DH], BF16, kind="ExternalInput")
    wk_t = nc.dram_tensor("wk", [128, KT * HPC * DH], BF16, kind="ExternalInput")
    wv_t = nc.dram_tensor("wv", [128, KT * HPC * DH], BF16, kind="ExternalInput")
    bq_t = nc.dram_tensor("bq", [DH, HPC], F32, kind="ExternalInput")
    bk_t = nc.dram_tensor("bk", [DH, HPC], F32, kind="ExternalInput")
    bv_t = nc.dram_tensor("bv", [1, HPC * DH], F32, kind="ExternalInput")
    wo_t = nc.dram_tensor("wo", [128, KT * CSL], BF16, kind="ExternalInput")
    bo_t = nc.dram_tensor("bo", [1, CSL], F32, kind="ExternalInput")
    mask_t = nc.dram_tensor("mask", [4 * 128, QG], BF16, kind="ExternalInput")
    y_t = nc.dram_tensor("y", [S, CSL], F32, kind="ExternalOutput")

    xT_r = xT_t.ap().rearrange("(kt p) s -> kt p s", p=128)
    mask_r = mask_t.ap().rearrange("(jm p) q -> jm p q", p=128)

    scale = 1.0 / float(np.sqrt(DH))

    with tile.TileContext(nc) as tc:
        with tc.tile_pool(name="const", bufs=1) as constp, \
             tc.tile_pool(name="prod", bufs=1) as prodp, \
             tc.tile_pool(name="dram", bufs=1, space="DRAM") as dramp:
            # Weight staging. Full-tensor DMAs only: column-sliced weight
            # loads fragment into 128x512B descriptors and crawl on a cold
            # queue. Sync carries wq(h0) then the x^T stream; everything else
            # rides the Scalar HWDGE queue in first-use order, with the
            # fragmented small transfers (biases, masks) last.
            wqk_sb = {}
            for nm_ in ("wq", "wk"):
                for hh in range(HPC):
                    wqk_sb[(nm_, hh)] = constp.tile(
                        [128, KT * DH], BF16,
                        tag=f"w_{nm_}{hh}", name=f"w_{nm_}{hh}")
            nc.sync.dma_start(wqk_sb[("wq", 0)][:], wq_t.ap()[:, 0:KT * DH])
            nc.scalar.dma_start(wqk_sb[("wk", 0)][:], wk_t.ap()[:, 0:KT * DH])
            nc.scalar.dma_start(wqk_sb[("wq", 1)][:],
                                wq_t.ap()[:, KT * DH:2 * KT * DH])
            nc.scalar.dma_start(wqk_sb[("wk", 1)][:],
                                wk_t.ap()[:, KT * DH:2 * KT * DH])
            wv_sb = constp.tile([128, KT * HPC * DH], BF16, tag="wv_sb",
                                name="wv_sb")
            wos = constp.tile([128, KT * CSL], BF16, tag="wos", name="wos")
            nc.scalar.dma_start(wv_sb[:], wv_t.ap()[:])
            nc.scalar.dma_start(wos[:], wo_t.ap()[:])
            bqs = constp.tile([DH, HPC], F32)
            nc.scalar.dma_start(bqs[:], bq_t.ap()[:])
            bks = constp.tile([DH, HPC], F32)
            nc.scalar.dma_start(bks[:], bk_t.ap()[:])
            masks = constp.tile([128, 4 * QG], BF16)
            for jm in range(4):
                nc.scalar.dma_start(masks[:, jm * QG:(jm + 1) * QG],
                                    mask_r[jm])
            bvs = constp.tile([128, HPC * DH], F32)
            bos = constp.tile([128, CSL], F32)
            nc.scalar.dma_start(
                bvs[:], bv_t.ap().to_broadcast((128, HPC * DH)))
            nc.scalar.dma_start(bos[:], bo_t.ap().to_broadcast((128, CSL)))
            ones_f32 = constp.tile([128, 128], F32)
            nc.vector.memset(ones_f32[:], 1.0)
            ones_col = constp.tile([128, 1], FP)
            nc.vector.tensor_copy(ones_col[:], ones_f32[:, 0:1])


            # --- products ---
            qkt = prodp.tile([128, HPC * S], FP)   # Q^T, head hh at cols hh*S
            kkt = prodp.tile([128, HPC * S], FP)   # K^T
            vt = prodp.tile([128, SBK * HPC * DH], FP)  # V, s-block sb at sb*256

            cin = [dramp.tile([HPC * DH, qw], BF16, tag=f"cin{g}",
                              name=f"cin{g}")
                   for g, (q0, qw, nj) in enumerate(GROUPS)]
            cout = [dramp.tile([N_CORES, HPC * DH, qw], BF16,
                               tag=f"cout{g}", name=f"cout{g}",
                               addr_space="Shared")
                    for g, (q0, qw, nj) in enumerate(GROUPS)]

            psum_stack = ExitStack()
            with tc.tile_pool(name="osb", bufs=1) as osbp, \
                 tc.tile_pool(name="pt", bufs=8) as ptp, \
                 tc.tile_pool(name="rec", bufs=2) as recp, \
                 tc.tile_pool(name="bcs", bufs=2) as bcsp:
                o_sbuf = osbp.tile([128, HPC * S], BF16)

                pp = {}   # persistent PSUM pools, filled after the stream

                def attn(gi, split_at=None, between=None):
                    q0, qw, nj = GROUPS[gi]
                    npairs = nj // 2
                    jmax = nj - 1
                    mask_start = q0 // 128     # first masked j-block
                    jm0 = (512 * (q0 // 512)) // 128
                    col_off = q0 - 512 * (q0 // 512)
                    psS, psO, psN = pp["psS"], pp["psO"], pp["psN"]
                    o_acc = [psO.tile([128, QG], F32, tag="o",
                                      name=f"o{hh}") for hh in range(HPC)]
                    s_acc = [psN.tile([1, QG], F32, tag="n",
                                      name=f"n{hh}") for hh in range(HPC)]

                    def emit_pv(hh, jp, pt):
                        for dj in range(2):
                            j = 2 * jp + dj
                            nc.tensor.matmul(
                                o_acc[hh][:, 0:qw],
                                vt[:, j * HPC * DH + hh * DH:
                                   j * HPC * DH + (hh + 1) * DH],
                                pt[:, dj * qw:(dj + 1) * qw],
                                start=(j == 0), stop=(j == jmax))
                            nc.tensor.matmul(
                                s_acc[hh][:, 0:qw],
                                ones_col[:],
                                pt[:, dj * qw:(dj + 1) * qw],
                                start=(j == 0), stop=(j == jmax))

                    recs = {}

                    pend = []

                    def pop_pend():
                        hh, jp, pt = pend.pop(0)
                        emit_pv(hh, jp, pt)
                        if jp == npairs - 1:
                            # reciprocal on DVE as soon as this head's
                            # denominator chain stops
                            rec = recp.tile([1, QG], F32, tag="r",
                                            name="rec")
                            nc.vector.reciprocal_approx_fast(
                                rec[:, 0:qw], s_acc[hh][:, 0:qw])
                            recs[hh] = rec

                    for jp in range(npairs):
                        if jp == split_at:
                            # flush, then emit interposed projection work;
                            # the PSUM accumulation chains stay open
                            while pend:
                                pop_pend()
                            between()
                        for hh in range(HPC):
                            pt = ptp.tile([128, 2 * QG], FP, tag="p",
                                          name="pt")
                            for dj in range(2):
                                j = 2 * jp + dj
                                ps = psS.tile([128, QG], F32, tag="s",
                                              name="ps")
                                nc.tensor.matmul(
                                    ps[:, 0:qw],
                                    kkt[:, hh * S + j * 128:
                                        hh * S + (j + 1) * 128],
                                    qkt[:, hh * S + q0:
                                        hh * S + q0 + qw],
                                    start=True, stop=True)
                                nc.scalar.activation(
                                    pt[:, dj * qw:(dj + 1) * qw],
                                    ps[:, 0:qw], AF.Exp, scale=scale)
                                if j >= mask_start:
                                    jm = j - jm0
                                    nc.vector.tensor_mul(
                                        pt[:, dj * qw:(dj + 1) * qw],
                                        pt[:, dj * qw:(dj + 1) * qw],
                                        masks[:, jm * QG + col_off:
                                              jm * QG + col_off + qw])
                            pend.append((hh, jp, pt))
                            while len(pend) > 3:
                                pop_pend()
                    while pend:
                        pop_pend()

                    for hh in range(HPC):
                        bcs = bcsp.tile([128, QG], F32, tag="b",
                                        name="bcs")
                        nc.gpsimd.partition_broadcast(
                            bcs[:, 0:qw], recs[hh][:, 0:qw])
                        nc.vector.tensor_mul(
                            o_sbuf[:, hh * S + q0:hh * S + q0 + qw],
                            o_acc[hh][:, 0:qw], bcs[:, 0:qw])
                        nc.sync.dma_start(
                            cin[gi].rearrange("(hh p) q -> hh p q",
                                              p=128)[hh],
                            o_sbuf[:, hh * S + q0:hh * S + q0 + qw])
                    nc.gpsimd.collective_compute(
                        "AllGather",
                        mybir.AluOpType.bypass,
                        replica_groups=[list(range(N_CORES))],
                        ins=[cin[gi].opt()],
                        outs=[cout[gi].opt()],
                    )

                # ===== interleaved projections + attention =====
                with tc.tile_pool(name="xt", bufs=1) as xtp:
                    xt = xtp.tile([128, KT * S], BF16)

                    def qk_specs(groups, heads=tuple(range(HPC))):
                        # drain order: everything attn(groups[0]) needs first
                        return [(wn, prod, bias, hh, g)
                                for g in groups
                                for hh in heads
                                for (wn, prod, bias) in
                                (("wq", qkt, bqs), ("wk", kkt, bks))]

                    def proj_stream(specs, tag):
                        """Stream x^T; QK chains from `specs` chase it."""
                        with tc.tile_pool(name=f"psP{tag}", bufs=1,
                                          space="PSUM") as psA:
                            accs = [psA.tile([128, QG], F32, tag="qk",
                                             name=f"qk{tag}_{i}",
                                             bufs=len(specs))
                                    for i in range(len(specs))]
                            for kt in range(KT):
                                # alternate queues so the stream never
                                # starves the PE (gpsimd is idle here)
                                eng = nc.sync if kt % 2 == 0 else nc.gpsimd
                                eng.dma_start(
                                    xt[:, kt * S:(kt + 1) * S], xT_r[kt])
                                for i, (wn, prod, bias, hh, g) in \
                                        enumerate(specs):
                                    nc.tensor.matmul(
                                        accs[i][:],
                                        wqk_sb[(wn, hh)][:, kt * DH:
                                                         (kt + 1) * DH],
                                        xt[:, kt * S + g * QG:
                                           kt * S + (g + 1) * QG],
                                        start=(kt == 0), stop=(kt == KT - 1))
                            for i, (wn, prod, bias, hh, g) in \
                                    enumerate(specs):
                                # alternate drains between Scalar and DVE to
                                # halve the serial drain tail
                                if i % 2 == 0:
                                    nc.scalar.activation(
                                        prod[:, hh * S + g * QG:
                                             hh * S + (g + 1) * QG],
                                        accs[i][:], AF.Identity,
                                        bias=bias[:, hh:hh + 1])
                                else:
                                    nc.vector.tensor_scalar_add(
                                        prod[:, hh * S + g * QG:
                                             hh * S + (g + 1) * QG],
                                        accs[i][:], bias[:, hh:hh + 1])

                    def proj_rest(specs, vbs, tag):
                        """QK chains from `specs` + V chains for blocks vbs,
                        resident x^T, chain-major so drains overlap. Accs
                        rotate through the persistent 2-slot acc pool."""
                        psA = pp["psA"]
                        for i, (wn, prod, bias, hh, g) in enumerate(specs):
                            acc = psA.tile([128, QG], F32, tag="acc",
                                           name=f"qk{tag}_{i}")
                            for kt in range(KT):
                                nc.tensor.matmul(
                                    acc[:],
                                    wqk_sb[(wn, hh)][:, kt * DH:
                                                     (kt + 1) * DH],
                                    xt[:, kt * S + g * QG:
                                       kt * S + (g + 1) * QG],
                                    start=(kt == 0), stop=(kt == KT - 1))
                            if i % 2 == 0:
                                nc.scalar.activation(
                                    prod[:, hh * S + g * QG:
                                         hh * S + (g + 1) * QG],
                                    acc[:], AF.Identity,
                                    bias=bias[:, hh:hh + 1])
                            else:
                                nc.vector.tensor_scalar_add(
                                    prod[:, hh * S + g * QG:
                                         hh * S + (g + 1) * QG],
                                    acc[:], bias[:, hh:hh + 1])
                        for bi, b in enumerate(vbs):
                            acc = psA.tile([128, QG], F32, tag="acc",
                                           name=f"v{tag}_{b}")
                            for kt in range(KT):
                                nc.tensor.matmul(
                                    acc[:, 0:HPC * DH],
                                    xt[:, kt * S + b * 128:
                                       kt * S + (b + 1) * 128],
                                    wv_sb[:, kt * HPC * DH:
                                          (kt + 1) * HPC * DH],
                                    start=(kt == 0), stop=(kt == KT - 1))
                            nc.vector.tensor_add(
                                vt[:, b * HPC * DH:(b + 1) * HPC * DH],
                                acc[:, 0:HPC * DH], bvs[:])

                    # stream pass: head-0 QK over all 4 groups, so the first
                    # matmul gates on only wq_h0 + wk_h0 + the first x tile
                    proj_stream(qk_specs([0, 1, 2, 3], heads=(0,)), "s0")
                    # persistent PSUM pools for the rest of the kernel: no
                    # pool-close barriers between phases (8 banks total)
                    pp["psS"] = psum_stack.enter_context(
                        tc.tile_pool(name="psS", bufs=2, space="PSUM"))
                    pp["psO"] = psum_stack.enter_context(
                        tc.tile_pool(name="psO", bufs=2, space="PSUM"))
                    pp["psN"] = psum_stack.enter_context(
                        tc.tile_pool(name="psN", bufs=2, space="PSUM"))
                    pp["psA"] = psum_stack.enter_context(
                        tc.tile_pool(name="psA", bufs=2, space="PSUM"))
                    proj_rest(qk_specs([0, 1], heads=(1,)),
                              [0, 1, 2, 3, 4, 5, 6, 7], "s0b")
                    attn(0)
                    proj_rest(qk_specs([2], heads=(1,)),
                              [8, 9, 10, 11], "s1")
                    attn(1)
                    attn(2)
                    proj_rest(qk_specs([3], heads=(1,)),
                              [12, 13, 14, 15], "s2")
                    attn(3)
                # xt pool closed: 64KB/partition freed for og prefetch
                with tc.tile_pool(name="og", bufs=4) as ogp:
                    og = {}

                    def og_load(gi):
                        q0, qw, nj = GROUPS[gi]
                        t = ogp.tile([128, KT * QG], BF16, tag="og",
                                     name=f"og{gi}")
                        og[gi] = t
                        # kt-quarters so the tail chain starts on the first
                        # quarter while the rest transfers. Sync queue: its
                        # only later work is the y stores, which trail the
                        # og-gated tails anyway.
                        Q4 = KT * qw // 4
                        for ci in range(4):
                            lo, hi = ci * Q4, (ci + 1) * Q4
                            nc.sync.dma_start(
                                t[:, lo:hi].rearrange(
                                    "p (c h q) -> p c h q",
                                    c=N_CORES // 4, h=HPC),
                                cout[gi].rearrange(
                                    "c (h p) q -> p c h q",
                                    p=128)[:, lo // (HPC * qw):
                                           hi // (HPC * qw)])

                    for gi in range(NG):
                        og_load(gi)

                    # ===== tail: output projection, column-sharded =====
                    with tc.tile_pool(name="yst", bufs=2) as ystp:
                        for gi, (q0, qw, nj) in enumerate(GROUPS):
                            nsb = qw // 128
                            yst = ystp.tile([128, 4 * CSL], F32, tag="ys",
                                            name=f"yst{gi}")
                            for i in range(nsb):
                                acc = pp["psA"].tile([128, QG], F32,
                                                     tag="acc", name="yacc")
                                for kt in range(KT):
                                    nc.tensor.matmul(
                                        acc[:, 0:CSL],
                                        og[gi][:, kt * qw + i * 128:
                                               kt * qw + (i + 1) * 128],
                                        wos[:, kt * CSL:(kt + 1) * CSL],
                                        start=(kt == 0), stop=(kt == KT - 1))
                                nc.vector.tensor_add(
                                    yst[:, i * CSL:(i + 1) * CSL],
                                    acc[:, 0:CSL], bos[:])
                            nc.sync.dma_start(
                                y_t.ap()[q0:q0 + qw, :].rearrange(
                                    "(sb p) c -> p sb c", p=128),
                                yst[:, 0:nsb * CSL].rearrange(
                                    "p (sb c) -> p sb c", c=CSL))
                psum_stack.close()

    nc.compile()
    return nc


def _tilize(w):
    """[E, cols] -> [128, KT*cols]: k-tile kt at columns kt*cols."""
    cols = w.shape[1]
    return np.ascontiguousarray(
        w.reshape(KT, 128, cols).transpose(1, 0, 2).reshape(128, KT * cols))


def _tilize_hm(w):
    """[E, HPC*DH] -> [128, HPC*KT*DH], head-major then k-tile."""
    return np.ascontiguousarray(
        w.reshape(KT, 128, HPC, DH).transpose(1, 2, 0, 3)
        .reshape(128, HPC * KT * DH))


def _prep_inputs(x, Wq, bq, Wk, bk, Wv, bv, WO, bo):
    import ml_dtypes

    f32 = np.float32
    bf16 = ml_dtypes.bfloat16
    xT = np.ascontiguousarray(np.asarray(x, f32)[0].T).astype(bf16)
    Wq = np.asarray(Wq, f32); Wk = np.asarray(Wk, f32); Wv = np.asarray(Wv, f32)
    bq = np.asarray(bq, f32); bk = np.asarray(bk, f32); bv = np.asarray(bv, f32)
    WO = np.asarray(WO, f32); bo = np.asarray(bo, f32)

    jm = np.arange(4)[:, None, None]
    r = np.arange(128)[None, :, None]
    c = np.arange(QG)[None, None, :]
    mask = (128 * jm + r <= c).astype(bf16).reshape(4 * 128, QG)

    in_maps = []
    for cidx in range(N_CORES):
        h0, h1 = HPC * cidx, HPC * cidx + 1
        in_maps.append({
            "xT": xT,
            "wq": _tilize_hm(np.concatenate([Wq[h0], Wq[h1]], 1)).astype(bf16),
            "wk": _tilize_hm(np.concatenate([Wk[h0], Wk[h1]], 1)).astype(bf16),
            "wv": _tilize(np.concatenate([Wv[h0], Wv[h1]], 1)).astype(bf16),
            "bq": np.ascontiguousarray(np.stack([bq[h0], bq[h1]], 1)),
            "bk": np.ascontiguousarray(np.stack([bk[h0], bk[h1]], 1)),
            "bv": np.concatenate([bv[h0], bv[h1]])[None, :].copy(),
            "wo": _tilize(np.ascontiguousarray(
                WO[:, CSL * cidx:CSL * (cidx + 1)])).astype(bf16),
            "bo": bo[CSL * cidx:CSL * (cidx + 1)][None, :].copy(),
            "mask": mask,
        })
    return in_maps


def kernel(x, Wq, bq, Wk, bk, Wv, bv, WO, bo, trace=False,
           fp_name="bfloat16"):
    from concourse.bass_utils import run_bass_kernel_spmd

    key = fp_name
    if key not in _CACHE:
        _CACHE[key] = _build(fp_name)
    nc = _CACHE[key]

    in_maps = _prep_inputs(x, Wq, bq, Wk, bk, Wv, bv, WO, bo)
    kwargs = {}
    if trace:
        kwargs["trace"] = True
    res = run_bass_kernel_spmd(nc, in_maps, core_ids=list(range(N_CORES)),
                               **kwargs)
    kernel.last_results = res

    y = np.concatenate([res.results[c]["y"] for c in range(N_CORES)], axis=1)
    return y.reshape(B, S, E).astype(np.float32)
